# revision 13
# baseline (speedup 1.0000x reference)
"""AttentionBlock (GroupNorm + 4-head self-attention + proj + residual) on 8 TRN2 cores.

Sharding: core = 2*b + hh  (b = batch 0..3, hh = head-half 0..1).
Each core handles one batch image and 2 of the 4 heads.

Engine-level structure:
 - The two heads' score matmuls (K=64 contraction each) are issued adjacently so
   the PE runs them concurrently via row tiling (partitions 0:64 / 64:128), into
   one [128, 2, 512] two-bank PSUM tile.
 - The softmax exp (N^2 logits/head, the elementwise bottleneck) alternates by
   k-chunk between ScalarE (exact Exp LUT) and VectorE (Schraudolph approximate
   exp: int16(S*a+b) whose bits, read as bf16, equal C*2^(S*scale*log2e); sigma
   is chosen so the approximation is unbiased against the exact chunks).
   One 1024-wide instruction covers both heads.
 - Paired-j emission: two score pairs back-to-back, then four attnv matmuls in
   a row, so PE weight-load/drain overlap breaks only twice per two k-chunks.
 - k bias is dropped (constant per query -> cancels in softmax); q bias is fused
   into the ScalarE PSUM->SBUF copy; v bias is folded into the output on host.
 - flash-style combine on host: the device ships raw per-head proj partials
   [256, N] and per-head softmax denominators [2, N]; the host computes
   out = sum_heads partial/denominator + const + residual.
 - x is loaded as bf16 (device only needs it for GroupNorm/qkv; the residual is
   added on host in fp32).
"""

import sys

sys.path.insert(0, "/opt/trn_rl_repo")

import numpy as np  # noqa: E402

import concourse.bacc as bacc  # noqa: E402
import concourse.tile as tile  # noqa: E402
from concourse import mybir  # noqa: E402
from concourse.bass_utils import run_bass_kernel_spmd  # noqa: E402

F32 = mybir.dt.float32
BF16 = mybir.dt.bfloat16
I16 = mybir.dt.int16
AF = mybir.ActivationFunctionType
ALU = mybir.AluOpType

# Problem constants (hardcoded per contract)
B, C, H, W = 4, 256, 64, 64
N = H * W          # 4096 pixels
NH, HD = 4, 64     # heads, head dim
GROUPS = 8
EPS = 1e-5
SCALE = HD ** -0.5  # 0.125

NCHUNK = 512            # pixel chunk (matmul moving dim)
NCH = N // NCHUNK       # 8
MCH = N // 128          # 32 k-chunks of 128 pixels

# Schraudolph exp-as-bf16-bits constants (DVE rounds to nearest; verified on HW)
LOG2E = 1.4426950408889634
A_C = SCALE * LOG2E * 128.0        # 23.0831...
# sigma centers the mean multiplicative ratio at 1.0 (the approx chunks mix
# with exact-exp chunks inside one softmax, so the constant must not bias)
B_C = 128.0 * (127.0 - 0.05641)    # 16248.78


def build_bass():
    nc = bacc.Bacc("TRN2", target_bir_lowering=False, debug=False)

    # ---- DRAM I/O (per-core shards fed via in_maps) ----
    xd = nc.dram_tensor("x", [C, N], BF16, kind="ExternalInput")
    # bf16 blob: wqT(2x128) | wkT(2x128) | wvT(2x130) | pwT(256) = 1028 cols
    wb_d = nc.dram_tensor("wb", [128, 1028], BF16, kind="ExternalInput")
    # f32 blob: qb(1) | nw(2) | nb(2) | indf(2x8) = 21 cols
    fb_d = nc.dram_tensor("fb", [128, 21], F32, kind="ExternalInput")
    indb_d = nc.dram_tensor("indb", [2, 8, 128], F32, kind="ExternalInput")
    outA_d = nc.dram_tensor("outA", [C, N], F32, kind="ExternalOutput")
    outB_d = nc.dram_tensor("outB", [C, N], F32, kind="ExternalOutput")
    sums_d = nc.dram_tensor("sums", [2, N], F32, kind="ExternalOutput")

    with tile.TileContext(nc) as tc:
        with (
            tc.tile_pool(name="persist", bufs=1) as pp,
            tc.tile_pool(name="tmp", bufs=4) as tp,
            tc.tile_pool(name="small", bufs=4) as sp,
            tc.tile_pool(name="a0pool", bufs=4) as a0p,
            tc.tile_pool(name="a1pool", bufs=4) as a1p,
            tc.tile_pool(name="onpool", bufs=2) as onp,
            tc.tile_pool(name="osbpool", bufs=4) as obp,
            tc.tile_pool(name="ps_sc", bufs=3, space="PSUM") as ps_sc,
            tc.tile_pool(name="ps_po", bufs=1, space="PSUM") as ps_po,
        ):
            # ================= Phase 0: loads & constants =================
            # x as bf16: 2 big DMAs per tile, split across both DMA queues
            x_t = []
            for i in range(2):
                xt = pp.tile([128, N], BF16, tag=f"x{i}", name=f"x{i}")
                eng = nc.sync if i == 0 else nc.scalar
                for lo, hi in ((0, 1024), (1024, 2048), (2048, 4096)):
                    eng.dma_start(
                        out=xt[:, lo:hi],
                        in_=xd[128 * i:128 * (i + 1), lo:hi])
                x_t.append(xt)

            # consolidated weight blobs (one DMA each)
            wb_t = pp.tile([128, 1028], BF16, tag="wb", name="wb")
            nc.scalar.dma_start(out=wb_t, in_=wb_d[:, :])
            fb_t = sp.tile([128, 21], F32, tag="fb", name="fb")
            nc.sync.dma_start(out=fb_t, in_=fb_d[:, :])
            wqT_t = [wb_t[:, 0:128], wb_t[:, 128:256]]
            wkT_t = [wb_t[:, 256:384], wb_t[:, 384:512]]
            wvT_t = [wb_t[:, 512:642], wb_t[:, 642:772]]
            pwT_t = wb_t[:, 772:1028]
            qb_t = fb_t[:, 0:1]
            nw_t = [fb_t[:, 1:2], fb_t[:, 2:3]]
            nb_t = [fb_t[:, 3:4], fb_t[:, 4:5]]
            indf_t = [fb_t[:, 5:13], fb_t[:, 13:21]]
            indb_t = []
            for i in range(2):
                t4 = sp.tile([8, 128], F32, tag=f"indb{i}", name=f"indb{i}")
                nc.sync.dma_start(out=t4, in_=indb_d[i])
                indb_t.append(t4)

            eps8 = sp.tile([8, 1], F32, tag="eps8", name="eps8")
            nc.vector.memset(eps8, EPS)
            # first ACT instruction is an Exp so the exp table set loads once,
            # early; Copy/Identity live in every set
            dummy = sp.tile([1, 1], BF16, tag="dummy", name="dummy")
            nc.scalar.activation(out=dummy, in_=eps8[0:1, 0:1], func=AF.Exp)

            # v_all[:, j, :] = [vA(64) | onesA(1) | vB(64) | onesB(1)]
            v_all = pp.tile([128, MCH, 130], BF16, tag="v_all", name="v_all")

            # ================= Phase 1: GroupNorm =================
            SDIM = nc.vector.BN_STATS_DIM   # 6
            ADIM = nc.vector.BN_AGGR_DIM    # 2
            NSUB = N // nc.vector.BN_STATS_FMAX if N > nc.vector.BN_STATS_FMAX else 1
            SUBLEN = N // NSUB

            # keep the PE active through the bn_stats stretch so HAM stays
            # unthrottled when the qkv matmuls arrive
            warm_ps = ps_sc.tile([128, 2, NCHUNK], F32, tag="sc", name="warm")
            for w in range(22):
                nc.tensor.matmul(warm_ps[0:32, 0, :], lhsT=x_t[0][:, 0:32],
                                 rhs=x_t[0][:, 0:NCHUNK], start=True, stop=True)

            m1e2 = []
            for i in range(2):
                st = tp.tile([128, NSUB, SDIM], F32, tag="bnst", name=f"bnst{i}")
                for s in range(NSUB):
                    nc.vector.bn_stats(
                        out=st[:, s, :],
                        in_=x_t[i][:, SUBLEN * s:SUBLEN * (s + 1)],
                    )
                mv = tp.tile([128, ADIM], F32, tag="bnmv", name=f"bnmv{i}")
                nc.vector.bn_aggr(out=mv, in_=st)
                me = sp.tile([128, 2], F32, tag=f"m1e2_{i}", name=f"m1e2_{i}")
                msq = tp.tile([128, 1], F32, tag="msq", name=f"msq{i}")
                nc.vector.tensor_mul(out=msq, in0=mv[:, 0:1], in1=mv[:, 0:1])
                nc.vector.tensor_copy(out=me[:, 0:1], in_=mv[:, 0:1])
                nc.vector.tensor_add(out=me[:, 1:2], in0=mv[:, 1:2], in1=msq)
                m1e2.append(me)

            psg_t = ps_sc.tile([128, 2, NCHUNK], F32, tag="sc", name="psg")
            psg = psg_t[0:8, 0, 0:2]
            nc.tensor.matmul(psg, lhsT=indf_t[0], rhs=m1e2[0], start=True, stop=False)
            nc.tensor.matmul(psg, lhsT=indf_t[1], rhs=m1e2[1], start=False, stop=True)

            sg = sp.tile([8, 2], F32, tag="sg", name="sg")
            nc.scalar.mul(out=sg, in_=psg, mul=1.0 / 32.0)  # [mean_g, e2_g]
            vg = sp.tile([8, 1], F32, tag="vg", name="vg")
            nc.vector.tensor_mul(out=vg, in0=sg[:, 0:1], in1=sg[:, 0:1])
            nc.vector.tensor_sub(out=vg, in0=sg[:, 1:2], in1=vg)  # var_g
            nc.vector.tensor_add(out=vg, in0=vg, in1=eps8)        # + eps
            # rstd via bit-trick seed + 2 Newton iterations, all on VectorE
            # (avoids loading the ScalarE sqrt table set: exp stays resident)
            ri = sp.tile([8, 1], I16.__class__ and mybir.dt.int32, tag="ri", name="ri")
            nc.vector.tensor_scalar(out=ri, in0=vg.bitcast(mybir.dt.int32),
                                    scalar1=1, scalar2=None,
                                    op0=ALU.arith_shift_right)
            rf = sp.tile([8, 1], F32, tag="rf", name="rf")
            nc.vector.tensor_scalar(out=rf, in0=ri, scalar1=-1.0,
                                    scalar2=1597463007.0,
                                    op0=ALU.mult, op1=ALU.add)
            nc.vector.tensor_copy(out=ri, in_=rf)
            r0 = ri.bitcast(F32)
            t1 = sp.tile([8, 1], F32, tag="t1", name="t1")
            for _ in range(2):
                nc.vector.tensor_mul(out=t1, in0=r0, in1=r0)
                nc.vector.tensor_mul(out=t1, in0=t1, in1=vg)
                nc.vector.tensor_scalar(out=t1, in0=t1, scalar1=-0.5,
                                        scalar2=1.5, op0=ALU.mult, op1=ALU.add)
                nc.vector.tensor_mul(out=ri.bitcast(F32), in0=r0, in1=t1)
            nc.vector.tensor_copy(out=sg[:, 1:2], in_=r0)         # rstd_g

            h_t, scoff = [], []
            for i in range(2):
                psc_t = ps_sc.tile([128, 2, NCHUNK], F32, tag="sc", name=f"psc{i}")
                psc = psc_t[:, 0, 0:2]
                nc.tensor.matmul(psc, lhsT=indb_t[i], rhs=sg, start=True, stop=True)
                sc = sp.tile([128, 1], F32, tag=f"sc{i}", name=f"sc{i}")
                off = sp.tile([128, 1], F32, tag=f"off{i}", name=f"off{i}")
                nc.vector.tensor_mul(out=sc, in0=psc[:, 1:2], in1=nw_t[i])
                nc.vector.tensor_mul(out=off, in0=psc[:, 0:1], in1=sc)
                nc.vector.tensor_sub(out=off, in0=nb_t[i], in1=off)
                ht = pp.tile([128, N], BF16, tag=f"h{i}", name=f"h{i}")
                h_t.append(ht)
                scoff.append((sc, off))

            # h = x*sc + off, chunks alternating ScalarE / VectorE
            for i in range(2):
                sc, off = scoff[i]
                for c4 in range(4):
                    csl = slice(1024 * c4, 1024 * (c4 + 1))
                    if (2 * i + c4) % 2 == 0:
                        nc.scalar.activation(
                            out=h_t[i][:, csl], in_=x_t[i][:, csl],
                            func=AF.Identity, bias=off, scale=sc)
                    else:
                        nc.vector.tensor_scalar(
                            out=h_t[i][:, csl], in0=x_t[i][:, csl],
                            scalar1=sc, scalar2=off, op0=ALU.mult, op1=ALU.add)

            # ================= Phase 2: k, v (q mostly deferred) ==========
            qT = pp.tile([128, N], BF16, tag="qT", name="qT")
            kT = pp.tile([128, N], BF16, tag="kT", name="kT")

            def emit_q(n):
                nsl = slice(NCHUNK * n, NCHUNK * (n + 1))
                psq = ps_sc.tile([128, 2, NCHUNK], F32, tag="sc", name=f"q{n}")
                nc.tensor.matmul(psq[:, 0, :], lhsT=wqT_t[0], rhs=h_t[0][:, nsl],
                                 start=True, stop=False)
                nc.tensor.matmul(psq[:, 0, :], lhsT=wqT_t[1], rhs=h_t[1][:, nsl],
                                 start=False, stop=True)
                nc.scalar.activation(out=qT[:, nsl], in_=psq[:, 0, :],
                                     func=AF.Identity, bias=qb_t, scale=1.0)

            for n in range(NCH):
                nsl = slice(NCHUNK * n, NCHUNK * (n + 1))
                psk = ps_sc.tile([128, 2, NCHUNK], F32, tag="sc", name=f"k{n}")
                nc.tensor.matmul(psk[:, 0, :], lhsT=wkT_t[0], rhs=h_t[0][:, nsl],
                                 start=True, stop=False)
                nc.tensor.matmul(psk[:, 0, :], lhsT=wkT_t[1], rhs=h_t[1][:, nsl],
                                 start=False, stop=True)
                if n % 2 == 0:
                    nc.vector.tensor_copy(out=kT[:, nsl], in_=psk[:, 0, :])
                else:
                    nc.scalar.copy(out=kT[:, nsl], in_=psk[:, 0, :])
                # v in [pixel, d] layout: two 130-wide chunks per 2-bank tile,
                # one 3D copy per pair
                for vp in range(2):
                    j0 = 4 * n + 2 * vp
                    psv_t = ps_sc.tile([128, 2, NCHUNK], F32, tag="sc",
                                       name=f"v{j0}")
                    for u in range(2):
                        j = j0 + u
                        psl = slice(128 * j, 128 * (j + 1))
                        nc.tensor.matmul(psv_t[:, u, 0:130], lhsT=h_t[0][:, psl],
                                         rhs=wvT_t[0], start=True, stop=False)
                        nc.tensor.matmul(psv_t[:, u, 0:130], lhsT=h_t[1][:, psl],
                                         rhs=wvT_t[1], start=False, stop=True)
                    if vp == 0:
                        nc.scalar.copy(out=v_all[:, j0:j0 + 2, :],
                                       in_=psv_t[:, :, 0:130])
                    else:
                        nc.vector.tensor_copy(out=v_all[:, j0:j0 + 2, :],
                                              in_=psv_t[:, :, 0:130])
                    nc.gpsimd.memset(v_all[:, j0:j0 + 2, 64:65], 1.0)
                    nc.gpsimd.memset(v_all[:, j0:j0 + 2, 129:130], 1.0)

            emit_q(0)
            emit_q(1)

            # ================= Phase 3: attention =================
            prev = None  # (po0, po1, n-1)

            def emit_tail_copies(po0, po1, pn):
                onrm = onp.tile([128, NCHUNK], BF16, tag="onrm", name=f"on{pn}")
                # denominators at partitions 0 (head A) and 64 (head B)
                sums = onp.tile([65, NCHUNK], F32, tag="sums", name=f"sm{pn}")
                nc.scalar.copy(out=onrm[0:64, :], in_=po0[0:64, :])
                nc.scalar.copy(out=sums[0:1, :], in_=po0[64:65, :])
                nc.vector.tensor_copy(out=onrm[64:128, :], in_=po1[0:64, :])
                nc.vector.tensor_copy(out=sums[64:65, :], in_=po1[64:65, :])
                return onrm, sums

            def emit_proj(onrm, ci, pn):
                csl = slice(128 * ci, 128 * (ci + 1))
                pj_t = ps_sc.tile([128, 2, NCHUNK], F32, tag="sc", name=f"pj{pn}_{ci}")
                pjA, pjB = pj_t[:, 0, :], pj_t[:, 1, :]
                nc.tensor.matmul(pjA, lhsT=pwT_t[0:64, csl], rhs=onrm[0:64, :],
                                 start=True, stop=True)
                nc.tensor.matmul(pjB, lhsT=pwT_t[64:128, csl], rhs=onrm[64:128, :],
                                 start=True, stop=True)
                return pjA, pjB

            def emit_osb(pjA, pjB, ci, pn):
                pnsl = slice(NCHUNK * pn, NCHUNK * (pn + 1))
                csl = slice(128 * ci, 128 * (ci + 1))
                oA = obp.tile([128, NCHUNK], F32, tag="osb", name=f"oA{pn}_{ci}")
                oB = obp.tile([128, NCHUNK], F32, tag="osb", name=f"oB{pn}_{ci}")
                nc.scalar.copy(out=oA, in_=pjA)
                if ci == 0:
                    nc.scalar.copy(out=oB, in_=pjB)
                else:
                    nc.vector.tensor_copy(out=oB, in_=pjB)
                nc.sync.dma_start(out=outA_d[csl, pnsl], in_=oA)
                nc.sync.dma_start(out=outB_d[csl, pnsl], in_=oB)

            for n in range(NCH):
                nsl = slice(NCHUNK * n, NCHUNK * (n + 1))
                tail = None
                if prev is not None:
                    tail = emit_tail_copies(*prev)

                po0 = ps_po.tile([65, NCHUNK], F32, tag="po0", name=f"po0_{n}")
                po1 = ps_po.tile([65, NCHUNK], F32, tag="po1", name=f"po1_{n}")
                ats = {}

                def emit_sc_exp(j):
                    jsl = slice(128 * j, 128 * (j + 1))
                    sAB = ps_sc.tile([128, 2, NCHUNK], F32, tag="sc",
                                     name=f"s{n}_{j}")
                    # adjacent K=64 matmuls on partition halves -> row-tiled,
                    # run concurrently on the PE
                    nc.tensor.matmul(sAB[:, 0, :], lhsT=kT[0:64, jsl],
                                     rhs=qT[0:64, nsl], start=True, stop=True)
                    nc.tensor.matmul(sAB[:, 1, :], lhsT=kT[64:128, jsl],
                                     rhs=qT[64:128, nsl], start=True, stop=True)
                    # one 1024-wide exp covering both heads, alternating engine
                    if j % 2 == 0:
                        at = a0p.tile([128, 2, NCHUNK], BF16, tag="a0",
                                      name=f"a_{n}_{j}")
                        nc.scalar.activation(out=at, in_=sAB, func=AF.Exp,
                                             scale=SCALE)
                    else:
                        at = a1p.tile([128, 2, NCHUNK], I16, tag="a1",
                                      name=f"a_{n}_{j}")
                        nc.vector.tensor_scalar(out=at, in0=sAB, scalar1=A_C,
                                                scalar2=B_C, op0=ALU.mult,
                                                op1=ALU.add)
                    ats[j] = at

                def emit_av(jj):
                    at = ats.pop(jj)
                    r0, r1 = at[:, 0, :], at[:, 1, :]
                    if jj % 2 == 1:
                        r0, r1 = r0.bitcast(BF16), r1.bitcast(BF16)
                    nc.tensor.matmul(po0, lhsT=v_all[:, jj, 0:65], rhs=r0,
                                     start=(jj == 0), stop=(jj == MCH - 1))
                    nc.tensor.matmul(po1, lhsT=v_all[:, jj, 65:130], rhs=r1,
                                     start=(jj == 0), stop=(jj == MCH - 1))

                # paired-j emission: 2 score pairs back-to-back, then 4 attnv
                # MMs in a row -- PE stream transitions (where LDW/drain
                # overlap breaks) drop from 4 to 2 per two k-chunks
                for jj in range(0, MCH, 2):
                    emit_sc_exp(jj)
                    emit_sc_exp(jj + 1)
                    if jj >= 4:
                        emit_av(jj - 4)
                        emit_av(jj - 3)
                    if tail is not None:
                        if jj == 4:
                            pj_state = emit_proj(tail[0], 0, n - 1)
                        elif jj == 6:
                            emit_osb(*pj_state, 0, n - 1)
                        elif jj == 10:
                            pj_state = emit_proj(tail[0], 1, n - 1)
                        elif jj == 12:
                            emit_osb(*pj_state, 1, n - 1)
                        elif jj == 14:
                            pnsl = slice(NCHUNK * (n - 1), NCHUNK * n)
                            nc.sync.dma_start(out=sums_d[:, pnsl],
                                              in_=tail[1][0:65:64, :])
                    if jj == 20 and n + 2 < NCH:
                        emit_q(n + 2)
                for jj in range(MCH - 4, MCH):
                    emit_av(jj)
                prev = (po0, po1, n)

            # final tail (n = NCH-1)
            tail = emit_tail_copies(*prev)
            pjA, pjB = emit_proj(tail[0], 0, NCH - 1)
            emit_osb(pjA, pjB, 0, NCH - 1)
            pjA, pjB = emit_proj(tail[0], 1, NCH - 1)
            emit_osb(pjA, pjB, 1, NCH - 1)
            lsl = slice(NCHUNK * (NCH - 1), NCHUNK * NCH)
            nc.sync.dma_start(out=sums_d[:, lsl], in_=tail[1][0:65:64, :])

    nc.compile()
    return nc


_NC_CACHE = None


def _get_nc():
    global _NC_CACHE
    if _NC_CACHE is None:
        _NC_CACHE = build_bass()
    return _NC_CACHE


def _bf16(a):
    import ml_dtypes
    return np.ascontiguousarray(a).astype(ml_dtypes.bfloat16)


def _make_in_maps(x, norm_w, norm_b, qkv_w, qkv_b, proj_w):
    ch = np.arange(128)
    indf = np.zeros((2, 128, 8), np.float32)
    indb = np.zeros((2, 8, 128), np.float32)
    for i in range(2):
        g = (i * 128 + ch) // 32
        indf[i, ch, g] = 1.0
        indb[i, g, ch] = 1.0
    nw = norm_w.reshape(2, 128, 1).astype(np.float32)
    nb = norm_b.reshape(2, 128, 1).astype(np.float32)

    in_maps = []
    for core in range(8):
        b, hh = core // 2, core % 2
        sl = slice(128 * hh, 128 * (hh + 1))
        wq = qkv_w[sl]                      # [128, 256]
        wk = qkv_w[256 + 128 * hh:256 + 128 * (hh + 1)]
        wv = qkv_w[512 + 128 * hh:512 + 128 * (hh + 1)]
        wvT = np.zeros((2, 128, 130), np.float32)
        for ci in range(2):
            csl = slice(128 * ci, 128 * (ci + 1))
            wvT[ci, :, 0:64] = wv[0:64, csl].T      # head A
            wvT[ci, :, 65:129] = wv[64:128, csl].T  # head B
        pwT = proj_w[:, sl].T                        # [128, 256]
        wb = np.concatenate([wq[:, 0:128].T, wq[:, 128:256].T,
                             wk[:, 0:128].T, wk[:, 128:256].T,
                             wvT[0], wvT[1], pwT], axis=1)  # [128, 1028]
        fb = np.concatenate([qkv_b[sl].reshape(128, 1), nw[0], nw[1],
                             nb[0], nb[1], indf[0], indf[1]],
                            axis=1).astype(np.float32)      # [128, 21]
        in_maps.append({
            "x": _bf16(x[b].reshape(C, N)),
            "wb": _bf16(wb),
            "fb": fb,
            "indb": indb,
        })
    return in_maps


def kernel(x, norm_w, norm_b, qkv_w, qkv_b, proj_w, proj_b, _trace=False, _tmpdir=None):
    x = np.asarray(x, np.float32)
    norm_w = np.asarray(norm_w, np.float32)
    norm_b = np.asarray(norm_b, np.float32)
    qkv_w = np.asarray(qkv_w, np.float32)
    qkv_b = np.asarray(qkv_b, np.float32)
    proj_w = np.asarray(proj_w, np.float32)
    proj_b = np.asarray(proj_b, np.float32)

    nc = _get_nc()
    in_maps = _make_in_maps(x, norm_w, norm_b, qkv_w, qkv_b, proj_w)
    kw = {}
    if _trace:
        kw = dict(trace=True, tmpdir=_tmpdir)
    res = run_bass_kernel_spmd(nc, in_maps, list(range(8)), **kw)

    # host: flash-style combine of per-head partials + residual
    vbias = qkv_b[512:768]
    const = (proj_w @ vbias + proj_b)[:, None].astype(np.float32)
    out = np.empty((B, C, H, W), np.float32)
    for b in range(B):
        acc = const + x[b].reshape(C, N)
        for hh in range(2):
            r = res.results[2 * b + hh]
            acc = acc + r["outA"] / r["sums"][0:1, :]
            acc = acc + r["outB"] / r["sums"][1:2, :]
        out[b] = acc.reshape(C, H, W)
    if _trace:
        return out, res
    return out
